# revision 8
# baseline (speedup 1.0000x reference)
"""Trainium2 Bass kernel for nn_MultiHeadAttention_32238024524256.

Multi-head attention (12 heads, d=768, T=2112=2048 x + 64 learnable queries)
with block-structured dropout on the attention matrix (training mode, fixed
threefry key), sharded over 8 NeuronCores: data-parallel over batch (2) x
tensor-parallel over heads (12) -> 24 (b,h) units, 3 per core.

Per-core dataflow (everything in "transposed" orientation so no on-chip
transposes are needed):
  inpT [768, 2176]  (d_in on partitions, seq padded 2112->17*128)
  QT/KT [64, T]  <- matmul(lhsT=[Wq_h|Wk_h], rhs=inpT)   (packed pair)
  V    [k, 64]   <- matmul(lhsT=inpT_tile, rhs=Wv_cols)  (natural layout)
  sT   [k, q]    <- matmul(lhsT=KT_tile, rhs=QT_chunk)   (scores^T)
  wE   = exp(sT/8)            on ACT (PSUM->SBUF, bf16; no max-subtraction --
                              scores are O(1) for this input distribution)
  wEm  = wE * maskT           on DVE (mask uint8 in HBM, cast->bf16 by SWDGE DMA)
  ctxT'[65, q]   <- matmul(lhsT=[V|ones], rhs=wEm)       (row 64 = softmax denom)
  ctxN = ctxT'[0:64] * (1/denom broadcast)               (dropout scale folded
                                                          into Wo on host)
  yp   [T, 768]  <- matmul(lhsT=ctxN_heads, rhs=Wo_rows) accumulated over the
                    core's 3 heads; host sums the 4 cores of each batch + bo.

The dropout mask is bit-exact with the reference: jax threefry on host CPU,
fixed key 42, packed per-core as uint8 [3, 2176, 2176] (k-major, padded k rows
are 0 so padding never contributes; padded q cols are 1 to avoid 0/0).
"""

import numpy as np

# ---------------------------------------------------------------- constants
B, NX, NLQ = 2, 2048, 64
D, H, HD = 768, 12, 64
T = NX + NLQ            # 2112
NKT = 17                # k tiles of 128
TP = NKT * 128          # 2176 padded
P_X, P_LQ = 0.1, 0.2
DROP_KEY = 42
N_CORES = 8
HPC = 3                 # (b,h) units per core

QCS = [(0, 512), (512, 512), (1024, 512), (1536, 512), (2048, 128)]
KT_PAIRS = [(0, 1), (2, 3), (4, 5), (6, 7), (8, 9), (10, 11), (12, 13),
            (14, 15), (16,)]

_SCALE = float((T * T) / (NX * NX * (1.0 - P_X) + (T * T - NX * NX) * (1.0 - P_LQ)))

# ---------------------------------------------------------------- host: mask
_MASK_CACHE = {}


def _mask_per_core():
    """uint8 maskT per core: [HPC, TP, TP], maskT[j][k, q] = keep(q, k) for the
    core's j-th (b,h). Padded k rows -> 0, padded q cols -> 1."""
    if "m" in _MASK_CACHE:
        return _MASK_CACHE["m"]
    import jax
    import jax.numpy as jnp

    # IMPORTANT: compute on the DEFAULT jax device (the axon/neuron backend),
    # exactly like the reference does -- this backend's threefry bits differ
    # from CPU's, and the mask must match the reference bit-for-bit.
    kp_x = 1.0 - P_X
    kp_lq = 1.0 - P_LQ
    is_x = jnp.arange(T) < NX
    keep_p = jnp.where(is_x[:, None] & is_x[None, :], kp_x, kp_lq)
    u = jax.random.uniform(jax.random.key(DROP_KEY), (B, H, T, T), jnp.float32)
    mask = np.asarray(u < keep_p)  # bool [B, H, T, T]
    per_core = []
    for c in range(N_CORES):
        arr = np.zeros((HPC, TP, TP), np.uint8)
        for j in range(HPC):
            f = c * HPC + j
            b, h = divmod(f, H)
            arr[j, :T, :T] = mask[b, h].T
            arr[j, :T, T:] = 1
        per_core.append(arr)
    _MASK_CACHE["m"] = per_core
    return per_core


# ---------------------------------------------------------------- bass build
_NC = None


def _build_nc(debug_dump=False):
    import concourse.mybir as mybir
    import concourse.tile as tile
    from concourse import bacc

    f32 = mybir.dt.float32
    bf16 = mybir.dt.bfloat16
    u8 = mybir.dt.uint8
    EXP = mybir.ActivationFunctionType.Exp

    nc = bacc.Bacc("TRN2", target_bir_lowering=False, debug=False)

    xT_d = nc.dram_tensor("xT", [D, TP], f32, kind="ExternalInput").ap()
    wqk_d = nc.dram_tensor("wqk", [HPC, D, 128], f32, kind="ExternalInput").ap()
    wv_d = nc.dram_tensor("wv", [D, HPC * HD], f32, kind="ExternalInput").ap()
    wo_d = nc.dram_tensor("wo", [HPC, HD, D], f32, kind="ExternalInput").ap()
    mk_d = nc.dram_tensor("maskT", [HPC, TP, TP], u8, kind="ExternalInput").ap()
    yp_d = nc.dram_tensor("yp", [T, D], f32, kind="ExternalOutput").ap()
    if debug_dump:
        dbg_qk = nc.dram_tensor("dbg_qk", [128, TP], f32, kind="ExternalOutput").ap()
        dbg_v3 = nc.dram_tensor("dbg_v3", [128, HPC, NKT, HD + 1], f32,
                                kind="ExternalOutput").ap()
        dbg_wem = nc.dram_tensor("dbg_wem", [128, 2, 512], f32,
                                 kind="ExternalOutput").ap()
        dbg_ctx = nc.dram_tensor("dbg_ctx", [65, 512], f32,
                                 kind="ExternalOutput").ap()
        dbg_ctxn = nc.dram_tensor("dbg_ctxn", [64, HPC, TP], f32,
                                  kind="ExternalOutput").ap()

    with tile.TileContext(nc) as tc:
        with tc.tile_pool(name="const", bufs=1) as const, \
             tc.tile_pool(name="qkpool", bufs=2) as qkpool, \
             tc.tile_pool(name="mkpool", bufs=2) as mkpool, \
             tc.tile_pool(name="wepool", bufs=2) as wepool, \
             tc.tile_pool(name="npool", bufs=2) as npool, \
             tc.tile_pool(name="opool", bufs=2) as opool, \
             tc.tile_pool(name="ps", bufs=1, space="PSUM") as ps:

            # ---- constants / inputs
            inpT = const.tile([128, 6, TP], f32)
            nc.sync.dma_start(out=inpT, in_=xT_d.rearrange("(a p) c -> p a c", p=128))
            wqk_s = const.tile([128, HPC, 6, 128], f32)
            nc.sync.dma_start(out=wqk_s,
                              in_=wqk_d.rearrange("h (a p) m -> p h a m", p=128))
            wv_s = const.tile([128, 6, HPC * HD], f32)
            nc.sync.dma_start(out=wv_s, in_=wv_d.rearrange("(a p) m -> p a m", p=128))
            wo_s = const.tile([64, HPC, D], f32)
            nc.sync.dma_start(out=wo_s, in_=wo_d.rearrange("h p n -> p h n"))
            v3 = const.tile([128, HPC, NKT, HD + 1], bf16)
            nc.vector.memset(v3[:, :, :, HD:HD + 1], 1.0)
            ctxN = const.tile([64, HPC, TP], f32)

            # ---- V projection for all 3 heads (natural [k, hd] layout)
            for kt in range(NKT):
                pv = ps.tile([128, HPC * HD], f32, tag="v", bufs=1)
                for d in range(6):
                    nc.tensor.matmul(out=pv,
                                     lhsT=inpT[:, d, kt * 128:(kt + 1) * 128],
                                     rhs=wv_s[:, d, :],
                                     start=(d == 0), stop=(d == 5))
                nc.vector.tensor_copy(v3[:, :, kt, 0:HD],
                                      pv.rearrange("p (h e) -> p h e", h=HPC))

            # ---- per-head attention
            for j in range(HPC):
                # QK projection: psum rows 0:64 = QT_h, rows 64:128 = KT_h
                qk = qkpool.tile([128, TP], f32, tag="qk", bufs=2)
                for (q0, qw) in QCS:
                    pqk = ps.tile([128, 512], f32, tag="qkp", bufs=1)
                    for d in range(6):
                        nc.tensor.matmul(out=pqk[:, :qw],
                                         lhsT=wqk_s[:, j, d, :],
                                         rhs=inpT[:, d, q0:q0 + qw],
                                         start=(d == 0), stop=(d == 5))
                    nc.vector.tensor_copy(qk[:, q0:q0 + qw], pqk[:, :qw])
                # KT copy to partitions 0:64 (scores lhsT must sit there)
                ktlo = qkpool.tile([64, TP], f32, tag="ktlo", bufs=1)
                nc.sync.dma_start(out=ktlo, in_=qk[64:128, :])
                if debug_dump and j == 0:
                    nc.sync.dma_start(out=dbg_qk, in_=qk)

                for (q0, qw) in QCS:
                    # dropout mask chunk for all 17 k-tiles, cast u8 -> bf16
                    mq = mkpool.tile([128, NKT, 512], bf16, tag="mq")
                    nc.gpsimd.dma_start(
                        out=mq[:, :, :qw],
                        in_=mk_d[j].rearrange("(k p) q -> p k q", p=128)[:, :, q0:q0 + qw])
                    pctx = ps.tile([65, 512], f32, tag="ctx", bufs=2)
                    for pi, pair in enumerate(KT_PAIRS):
                        npair = len(pair)
                        psc = ps.tile([128, 2, 512], f32, tag="s", bufs=2)
                        for i, kt in enumerate(pair):
                            nc.tensor.matmul(out=psc[:, i, :qw],
                                             lhsT=ktlo[:, kt * 128:(kt + 1) * 128],
                                             rhs=qk[0:64, q0:q0 + qw],
                                             start=True, stop=True)
                        wE = wepool.tile([128, 2, 512], bf16, tag="we")
                        nc.scalar.activation(out=wE[:, :npair, :qw],
                                             in_=psc[:, :npair, :qw],
                                             func=EXP, scale=0.125)
                        wEm = wepool.tile([128, 2, 512], bf16, tag="wem")
                        nc.vector.tensor_mul(wEm[:, :npair, :qw],
                                             wE[:, :npair, :qw],
                                             mq[:, 2 * pi:2 * pi + npair, :qw])
                        if debug_dump and j == 0 and q0 == 0 and pi == 0:
                            nc.gpsimd.dma_start(out=dbg_wem, in_=wEm)
                        for i, kt in enumerate(pair):
                            # last k-tile: only rows 0:64 are real tokens
                            kk = 128 if kt < NKT - 1 else 64
                            # ctx numerator: V^T @ (wE*mask) -> partitions 0:64
                            nc.tensor.matmul(out=pctx[0:HD, :qw],
                                             lhsT=v3[:kk, j, kt, 0:HD],
                                             rhs=wEm[:kk, i, :qw],
                                             start=(kt == 0), stop=(kt == NKT - 1))
                            # softmax denominator: ones^T @ wE (UNmasked; the
                            # reference applies dropout after softmax) ->
                            # partition 64, col-tiled so it runs concurrently
                            nc.tensor.matmul(out=pctx[HD:HD + 1, :qw],
                                             lhsT=v3[:kk, j, kt, HD:HD + 1],
                                             rhs=wE[:kk, i, :qw],
                                             start=(kt == 0), stop=(kt == NKT - 1),
                                             tile_position=(0, 64))
                    if debug_dump and j == 0 and q0 == 0:
                        dbg_ctx_sb = opool.tile([65, 512], f32, tag="dbgc", bufs=1)
                        nc.vector.tensor_copy(dbg_ctx_sb, pctx)
                        nc.sync.dma_start(out=dbg_ctx, in_=dbg_ctx_sb)
                    # normalize: ctxN = ctxT' * (1/denom), denom in row 64
                    # (copy denom to partition 0 first: reciprocal_approx_fast
                    # mishandles inputs at a non-zero base partition)
                    den = npool.tile([1, 512], f32, tag="den")
                    nc.vector.tensor_copy(den[:, :qw], pctx[64:65, :qw])
                    rec = npool.tile([1, 512], f32, tag="rec")
                    nc.vector.reciprocal_approx_fast(out=rec[:, :qw],
                                                     in_=den[:, :qw])
                    rb = npool.tile([64, 512], f32, tag="rb")
                    nc.gpsimd.partition_broadcast(rb[:, :qw], rec[:, :qw])
                    nc.vector.tensor_mul(ctxN[:, j, q0:q0 + qw],
                                         pctx[0:64, :qw], rb[:, :qw])

            if debug_dump:
                nc.gpsimd.dma_start(out=dbg_v3, in_=v3)
                nc.sync.dma_start(out=dbg_ctxn, in_=ctxN)

            # ---- output projection (scale already folded into wo on host)
            for tt in range(NKT):
                po = ps.tile([128, D], f32, tag="s", bufs=2)
                for j in range(HPC):
                    nc.tensor.matmul(out=po[:, 0:512],
                                     lhsT=ctxN[:, j, tt * 128:(tt + 1) * 128],
                                     rhs=wo_s[:, j, 0:512],
                                     start=(j == 0), stop=(j == HPC - 1))
                    nc.tensor.matmul(out=po[:, 512:D],
                                     lhsT=ctxN[:, j, tt * 128:(tt + 1) * 128],
                                     rhs=wo_s[:, j, 512:D],
                                     start=(j == 0), stop=(j == HPC - 1))
                ot = opool.tile([128, D], f32, tag="ot")
                nc.vector.tensor_copy(ot, po)
                rows = min(128, T - tt * 128)
                nc.sync.dma_start(out=yp_d[tt * 128: tt * 128 + rows, :],
                                  in_=ot[:rows, :])

    nc.compile()
    return nc


def _get_nc():
    global _NC
    if _NC is None:
        _NC = _build_nc()
    return _NC


# ---------------------------------------------------------------- runner
_RUNNER = {}


def _run_spmd(nc, in_maps):
    """First call goes through bass_utils.run_bass_kernel_spmd (which compiles
    the NEFF and runs via PJRT on the 8 cores); later calls reuse a cached
    jitted executable to avoid recompiling."""
    if "fn" not in _RUNNER:
        from concourse.bass_utils import run_bass_kernel_spmd
        res = run_bass_kernel_spmd(nc, in_maps, core_ids=list(range(N_CORES)))
        return res.results
    return _RUNNER["fn"](in_maps)


# ---------------------------------------------------------------- host pack
def _pack_inputs(x, lq, Wq, Wk, Wv, Wo):
    """Per-core input dicts (everything except the cached mask)."""
    Wos = (Wo.astype(np.float64) * _SCALE).astype(np.float32)
    xTs = []
    for b in range(B):
        inp = np.concatenate([x[b], lq[b]], axis=0)  # [T, D]
        xT = np.zeros((D, TP), np.float32)
        xT[:, :T] = np.ascontiguousarray(inp.T)
        xTs.append(xT)
    masks = _mask_per_core()
    in_maps = []
    for c in range(N_CORES):
        f0 = c * HPC
        b = f0 // H
        hs = [(f0 + j) % H for j in range(HPC)]
        wqk = np.empty((HPC, D, 128), np.float32)
        for j, h in enumerate(hs):
            wqk[j, :, 0:HD] = Wq[:, h * HD:(h + 1) * HD]
            wqk[j, :, HD:128] = Wk[:, h * HD:(h + 1) * HD]
        wv = np.concatenate([Wv[:, h * HD:(h + 1) * HD] for h in hs], axis=1)
        wo = np.stack([Wos[h * HD:(h + 1) * HD, :] for h in hs])
        in_maps.append({
            "xT": xTs[b],
            "wqk": np.ascontiguousarray(wqk),
            "wv": np.ascontiguousarray(wv),
            "wo": np.ascontiguousarray(wo),
            "maskT": masks[c],
        })
    return in_maps


# ---------------------------------------------------------------- entry
def kernel(x, learnable_queries, Wq, Wk, Wv, Wo, bo):
    x = np.asarray(x, np.float32)
    lq = np.asarray(learnable_queries, np.float32)
    Wq = np.asarray(Wq, np.float32)
    Wk = np.asarray(Wk, np.float32)
    Wv = np.asarray(Wv, np.float32)
    Wo = np.asarray(Wo, np.float32)
    bo = np.asarray(bo, np.float32)

    nc = _get_nc()
    in_maps = _pack_inputs(x, lq, Wq, Wk, Wv, Wo)
    results = _run_spmd(nc, in_maps)

    y = np.zeros((B, T, D), np.float32)
    for c in range(N_CORES):
        b = (c * HPC) // H
        y[b] += results[c]["yp"]
    y += bo
    return y


# revision 10
# speedup vs baseline: 17.3949x; 17.3949x over previous
"""Trainium2 Bass kernel for nn_MultiHeadAttention_32238024524256.

Multi-head attention (12 heads, d=768, T=2112=2048 x + 64 learnable queries)
with block-structured dropout on the attention matrix (training mode, fixed
threefry key), sharded over 8 NeuronCores: data-parallel over batch (2) x
tensor-parallel over heads (12) -> 24 (b,h) units, 3 per core.

Per-core dataflow (everything in "transposed" orientation so no on-chip
transposes are needed):
  inpT [768, 2176]  (d_in on partitions, seq padded 2112->17*128)
  QT/KT [64, T]  <- matmul(lhsT=[Wq_h|Wk_h], rhs=inpT)   (packed pair)
  V    [k, 64]   <- matmul(lhsT=inpT_tile, rhs=Wv_cols)  (natural layout)
  sT   [k, q]    <- matmul(lhsT=KT_tile, rhs=QT_chunk)   (scores^T)
  wE   = exp(sT/8)            on ACT (PSUM->SBUF, bf16; no max-subtraction --
                              scores are O(1) for this input distribution)
  wEm  = wE * maskT           on DVE (mask uint8 in HBM, cast->bf16 by SWDGE DMA)
  ctxT'[65, q]   <- matmul(lhsT=[V|ones], rhs=wEm)       (row 64 = softmax denom)
  ctxN = ctxT'[0:64] * (1/denom broadcast)               (dropout scale folded
                                                          into Wo on host)
  yp   [T, 768]  <- matmul(lhsT=ctxN_heads, rhs=Wo_rows) accumulated over the
                    core's 3 heads; host sums the 4 cores of each batch + bo.

The dropout mask is bit-exact with the reference: jax threefry on host CPU,
fixed key 42, packed per-core as uint8 [3, 2176, 2176] (k-major, padded k rows
are 0 so padding never contributes; padded q cols are 1 to avoid 0/0).
"""

import numpy as np

# ---------------------------------------------------------------- constants
B, NX, NLQ = 2, 2048, 64
D, H, HD = 768, 12, 64
T = NX + NLQ            # 2112
NKT = 17                # k tiles of 128
TP = NKT * 128          # 2176 padded
P_X, P_LQ = 0.1, 0.2
DROP_KEY = 42
N_CORES = 8
HPC = 3                 # (b,h) units per core

QCS = [(0, 512), (512, 512), (1024, 512), (1536, 512), (2048, 128)]
KT_PAIRS = [(0, 1), (2, 3), (4, 5), (6, 7), (8, 9), (10, 11), (12, 13),
            (14, 15), (16,)]

_SCALE = float((T * T) / (NX * NX * (1.0 - P_X) + (T * T - NX * NX) * (1.0 - P_LQ)))

# ---------------------------------------------------------------- host: mask
_MASK_CACHE = {}


def _mask_per_core():
    """uint8 maskT per core: [HPC, TP, TP], maskT[j][k, q] = keep(q, k) for the
    core's j-th (b,h). Padded k rows -> 0, padded q cols -> 1."""
    if "m" in _MASK_CACHE:
        return _MASK_CACHE["m"]
    import jax
    import jax.numpy as jnp

    # IMPORTANT: compute on the DEFAULT jax device (the axon/neuron backend),
    # exactly like the reference does -- this backend's threefry bits differ
    # from CPU's, and the mask must match the reference bit-for-bit.
    kp_x = 1.0 - P_X
    kp_lq = 1.0 - P_LQ
    is_x = jnp.arange(T) < NX
    keep_p = jnp.where(is_x[:, None] & is_x[None, :], kp_x, kp_lq)
    u = jax.random.uniform(jax.random.key(DROP_KEY), (B, H, T, T), jnp.float32)
    mask = np.asarray(u < keep_p)  # bool [B, H, T, T]
    per_core = []
    for c in range(N_CORES):
        arr = np.zeros((HPC, TP, TP), np.uint8)
        for j in range(HPC):
            f = c * HPC + j
            b, h = divmod(f, H)
            arr[j, :T, :T] = mask[b, h].T
            arr[j, :T, T:] = 1
        per_core.append(arr)
    _MASK_CACHE["m"] = per_core
    return per_core


# ---------------------------------------------------------------- bass build
_NC = None


def _build_nc(debug_dump=False, loop_n=None):
    import contextlib
    import concourse.mybir as mybir
    import concourse.tile as tile
    from concourse import bacc

    f32 = mybir.dt.float32
    bf16 = mybir.dt.bfloat16
    u8 = mybir.dt.uint8
    EXP = mybir.ActivationFunctionType.Exp

    nc = bacc.Bacc("TRN2", target_bir_lowering=False, debug=False)

    xT_d = nc.dram_tensor("xT", [D, TP], f32, kind="ExternalInput").ap()
    wqk_d = nc.dram_tensor("wqk", [HPC, D, 128], f32, kind="ExternalInput").ap()
    wv_d = nc.dram_tensor("wv", [D, HPC * HD], f32, kind="ExternalInput").ap()
    wo_d = nc.dram_tensor("wo", [HPC, HD, D], f32, kind="ExternalInput").ap()
    mk_d = nc.dram_tensor("maskT", [HPC, TP, TP], u8, kind="ExternalInput").ap()
    yp_d = nc.dram_tensor("yp", [T, D], f32, kind="ExternalOutput").ap()
    if debug_dump:
        dbg_qk = nc.dram_tensor("dbg_qk", [128, TP], f32, kind="ExternalOutput").ap()
        dbg_v3 = nc.dram_tensor("dbg_v3", [128, HPC, NKT, HD + 1], f32,
                                kind="ExternalOutput").ap()
        dbg_wem = nc.dram_tensor("dbg_wem", [128, 2, 512], f32,
                                 kind="ExternalOutput").ap()
        dbg_ctx = nc.dram_tensor("dbg_ctx", [65, 512], f32,
                                 kind="ExternalOutput").ap()
        dbg_ctxn = nc.dram_tensor("dbg_ctxn", [64, HPC, TP], f32,
                                  kind="ExternalOutput").ap()

    with tile.TileContext(nc) as tc:
        with tc.tile_pool(name="const", bufs=1) as const, \
             tc.tile_pool(name="qkpool", bufs=2) as qkpool, \
             tc.tile_pool(name="mkpool", bufs=2) as mkpool, \
             tc.tile_pool(name="wepool", bufs=2) as wepool, \
             tc.tile_pool(name="npool", bufs=2) as npool, \
             tc.tile_pool(name="opool", bufs=2) as opool, \
             tc.tile_pool(name="ps", bufs=1, space="PSUM") as ps, \
             (tc.For_i(0, loop_n, 1) if loop_n else contextlib.nullcontext()):

            # ---- constants / inputs
            inpT = const.tile([128, 6, TP], f32)
            nc.sync.dma_start(out=inpT, in_=xT_d.rearrange("(a p) c -> p a c", p=128))
            wqk_s = const.tile([128, HPC, 6, 128], f32)
            nc.sync.dma_start(out=wqk_s,
                              in_=wqk_d.rearrange("h (a p) m -> p h a m", p=128))
            wv_s = const.tile([128, 6, HPC * HD], f32)
            nc.sync.dma_start(out=wv_s, in_=wv_d.rearrange("(a p) m -> p a m", p=128))
            wo_s = const.tile([64, HPC, D], f32)
            nc.sync.dma_start(out=wo_s, in_=wo_d.rearrange("h p n -> p h n"))
            v3 = const.tile([128, HPC, NKT, HD + 1], bf16)
            nc.vector.memset(v3[:, :, :, HD:HD + 1], 1.0)
            ctxN = const.tile([64, HPC, TP], f32)

            # ---- V projection for all 3 heads (natural [k, hd] layout)
            for kt in range(NKT):
                pv = ps.tile([128, HPC * HD], f32, tag="v", bufs=1)
                for d in range(6):
                    nc.tensor.matmul(out=pv,
                                     lhsT=inpT[:, d, kt * 128:(kt + 1) * 128],
                                     rhs=wv_s[:, d, :],
                                     start=(d == 0), stop=(d == 5))
                nc.vector.tensor_copy(v3[:, :, kt, 0:HD],
                                      pv.rearrange("p (h e) -> p h e", h=HPC))

            # ---- per-head attention
            for j in range(HPC):
                # QK projection: psum rows 0:64 = QT_h, rows 64:128 = KT_h
                qk = qkpool.tile([128, TP], f32, tag="qk", bufs=2)
                for (q0, qw) in QCS:
                    pqk = ps.tile([128, 512], f32, tag="qkp", bufs=1)
                    for d in range(6):
                        nc.tensor.matmul(out=pqk[:, :qw],
                                         lhsT=wqk_s[:, j, d, :],
                                         rhs=inpT[:, d, q0:q0 + qw],
                                         start=(d == 0), stop=(d == 5))
                    nc.vector.tensor_copy(qk[:, q0:q0 + qw], pqk[:, :qw])
                # KT copy to partitions 0:64 (scores lhsT must sit there)
                ktlo = qkpool.tile([64, TP], f32, tag="ktlo", bufs=1)
                nc.sync.dma_start(out=ktlo, in_=qk[64:128, :])
                if debug_dump and j == 0:
                    nc.sync.dma_start(out=dbg_qk, in_=qk)

                for (q0, qw) in QCS:
                    # dropout mask chunk for all 17 k-tiles, cast u8 -> bf16
                    mq = mkpool.tile([128, NKT, 512], bf16, tag="mq")
                    nc.gpsimd.dma_start(
                        out=mq[:, :, :qw],
                        in_=mk_d[j].rearrange("(k p) q -> p k q", p=128)[:, :, q0:q0 + qw])
                    pctx = ps.tile([65, 512], f32, tag="ctx", bufs=2)
                    for pi, pair in enumerate(KT_PAIRS):
                        npair = len(pair)
                        psc = ps.tile([128, 2, 512], f32, tag="s", bufs=2)
                        for i, kt in enumerate(pair):
                            nc.tensor.matmul(out=psc[:, i, :qw],
                                             lhsT=ktlo[:, kt * 128:(kt + 1) * 128],
                                             rhs=qk[0:64, q0:q0 + qw],
                                             start=True, stop=True)
                        wE = wepool.tile([128, 2, 512], bf16, tag="we")
                        nc.scalar.activation(out=wE[:, :npair, :qw],
                                             in_=psc[:, :npair, :qw],
                                             func=EXP, scale=0.125)
                        wEm = wepool.tile([128, 2, 512], bf16, tag="wem")
                        nc.vector.tensor_mul(wEm[:, :npair, :qw],
                                             wE[:, :npair, :qw],
                                             mq[:, 2 * pi:2 * pi + npair, :qw])
                        if debug_dump and j == 0 and q0 == 0 and pi == 0:
                            nc.gpsimd.dma_start(out=dbg_wem, in_=wEm)
                        for i, kt in enumerate(pair):
                            # last k-tile: only rows 0:64 are real tokens
                            kk = 128 if kt < NKT - 1 else 64
                            # ctx numerator: V^T @ (wE*mask) -> partitions 0:64
                            nc.tensor.matmul(out=pctx[0:HD, :qw],
                                             lhsT=v3[:kk, j, kt, 0:HD],
                                             rhs=wEm[:kk, i, :qw],
                                             start=(kt == 0), stop=(kt == NKT - 1))
                            # softmax denominator: ones^T @ wE (UNmasked; the
                            # reference applies dropout after softmax) ->
                            # partition 64, col-tiled so it runs concurrently
                            nc.tensor.matmul(out=pctx[HD:HD + 1, :qw],
                                             lhsT=v3[:kk, j, kt, HD:HD + 1],
                                             rhs=wE[:kk, i, :qw],
                                             start=(kt == 0), stop=(kt == NKT - 1),
                                             tile_position=(0, 64))
                    if debug_dump and j == 0 and q0 == 0:
                        dbg_ctx_sb = opool.tile([65, 512], f32, tag="dbgc", bufs=1)
                        nc.vector.tensor_copy(dbg_ctx_sb, pctx)
                        nc.sync.dma_start(out=dbg_ctx, in_=dbg_ctx_sb)
                    # normalize: ctxN = ctxT' * (1/denom), denom in row 64
                    # (copy denom to partition 0 first: reciprocal_approx_fast
                    # mishandles inputs at a non-zero base partition)
                    den = npool.tile([1, 512], f32, tag="den")
                    nc.vector.tensor_copy(den[:, :qw], pctx[64:65, :qw])
                    rec = npool.tile([1, 512], f32, tag="rec")
                    nc.vector.reciprocal_approx_fast(out=rec[:, :qw],
                                                     in_=den[:, :qw])
                    rb = npool.tile([64, 512], f32, tag="rb")
                    nc.gpsimd.partition_broadcast(rb[:, :qw], rec[:, :qw])
                    nc.vector.tensor_mul(ctxN[:, j, q0:q0 + qw],
                                         pctx[0:64, :qw], rb[:, :qw])

            if debug_dump:
                nc.gpsimd.dma_start(out=dbg_v3, in_=v3)
                nc.sync.dma_start(out=dbg_ctxn, in_=ctxN)

            # ---- output projection (scale already folded into wo on host)
            for tt in range(NKT):
                po = ps.tile([128, D], f32, tag="s", bufs=2)
                for j in range(HPC):
                    nc.tensor.matmul(out=po[:, 0:512],
                                     lhsT=ctxN[:, j, tt * 128:(tt + 1) * 128],
                                     rhs=wo_s[:, j, 0:512],
                                     start=(j == 0), stop=(j == HPC - 1))
                    nc.tensor.matmul(out=po[:, 512:D],
                                     lhsT=ctxN[:, j, tt * 128:(tt + 1) * 128],
                                     rhs=wo_s[:, j, 512:D],
                                     start=(j == 0), stop=(j == HPC - 1))
                ot = opool.tile([128, D], f32, tag="ot")
                nc.vector.tensor_copy(ot, po)
                rows = min(128, T - tt * 128)
                nc.sync.dma_start(out=yp_d[tt * 128: tt * 128 + rows, :],
                                  in_=ot[:rows, :])

    nc.compile()
    return nc


def _get_nc():
    global _NC
    if _NC is None:
        _NC = _build_nc()
    return _NC


# ---------------------------------------------------------------- runner
_RUNNER = {}


def _run_spmd(nc, in_maps):
    """First call goes through bass_utils.run_bass_kernel_spmd (which compiles
    the NEFF and runs via PJRT on the 8 cores); later calls reuse a cached
    jitted executable to avoid recompiling."""
    if "fn" not in _RUNNER:
        from concourse.bass_utils import run_bass_kernel_spmd
        res = run_bass_kernel_spmd(nc, in_maps, core_ids=list(range(N_CORES)))
        return res.results
    return _RUNNER["fn"](in_maps)


# ---------------------------------------------------------------- host pack
def _pack_inputs(x, lq, Wq, Wk, Wv, Wo):
    """Per-core input dicts (everything except the cached mask)."""
    Wos = (Wo.astype(np.float64) * _SCALE).astype(np.float32)
    xTs = []
    for b in range(B):
        inp = np.concatenate([x[b], lq[b]], axis=0)  # [T, D]
        xT = np.zeros((D, TP), np.float32)
        xT[:, :T] = np.ascontiguousarray(inp.T)
        xTs.append(xT)
    masks = _mask_per_core()
    in_maps = []
    for c in range(N_CORES):
        f0 = c * HPC
        b = f0 // H
        hs = [(f0 + j) % H for j in range(HPC)]
        wqk = np.empty((HPC, D, 128), np.float32)
        for j, h in enumerate(hs):
            wqk[j, :, 0:HD] = Wq[:, h * HD:(h + 1) * HD]
            wqk[j, :, HD:128] = Wk[:, h * HD:(h + 1) * HD]
        wv = np.concatenate([Wv[:, h * HD:(h + 1) * HD] for h in hs], axis=1)
        wo = np.stack([Wos[h * HD:(h + 1) * HD, :] for h in hs])
        in_maps.append({
            "xT": xTs[b],
            "wqk": np.ascontiguousarray(wqk),
            "wv": np.ascontiguousarray(wv),
            "wo": np.ascontiguousarray(wo),
            "maskT": masks[c],
        })
    return in_maps


# ---------------------------------------------------------------- entry
def kernel(x, learnable_queries, Wq, Wk, Wv, Wo, bo):
    x = np.asarray(x, np.float32)
    lq = np.asarray(learnable_queries, np.float32)
    Wq = np.asarray(Wq, np.float32)
    Wk = np.asarray(Wk, np.float32)
    Wv = np.asarray(Wv, np.float32)
    Wo = np.asarray(Wo, np.float32)
    bo = np.asarray(bo, np.float32)

    nc = _get_nc()
    in_maps = _pack_inputs(x, lq, Wq, Wk, Wv, Wo)
    results = _run_spmd(nc, in_maps)

    y = np.zeros((B, T, D), np.float32)
    for c in range(N_CORES):
        b = (c * HPC) // H
        y[b] += results[c]["yp"]
    y += bo
    return y
